# revision 1
# baseline (speedup 1.0000x reference)
"""3-layer GAT on 8 trn2 NeuronCores — instruction-count-minimized design.

This stack has a ~70us fixed cost per engine instruction, so the kernel is
built around a few fat gpsimd DMA ops per ~3K edges instead of per-chunk
one-hot matmuls:

  per layer: project x->f table (f|el|ones cols) in DRAM; per edge batch:
  dma_gather f[src] rows, compute ex=exp(leaky(el_src+er_dst)) with ~8 fat
  DVE ops (er picked from an AllGathered er_full via a 64-wide one-hot
  select), scale rows by ex, dma_scatter_add into a per-core [50176, C]
  DRAM accumulator (row = pi(dst)); one ReduceScatter delivers summed rows
  (agg | z) to the dst owner; post normalizes (relu / log_softmax).

Scatter-add correctness: duplicate dst rows within one scatter race across
DMA rings, so edges are split into rounds with unique dst per round; calls
are serialized by the accumulator WAW dependency. int16 scatter indices cap
at 32767 -> the accumulator is addressed in two 25088-row halves.

Node order is "rho-space": local node n=(t*128+p) lives at table row
rho=p*49+t, which makes every DRAM<->SBUF layout partition-contiguous and
lets layers 2/3 load x^T with a single 2-byte dma transpose.
"""

import os
import numpy as np

N, E, DIN, DH, DOUT = 50000, 800000, 256, 128, 64
NCORES = 8
PC = 6272            # nodes per core
NPAD = PC * NCORES   # 50176
WPC = 49             # 128-row windows per core
P = 128
HALF = NPAD // 2     # 25088 (< int16 max)
GMAX = 30            # chunks (128 slots) per batch
TC12, TC3 = 192, 128  # table/acc row widths (f32)


def _wrap16(idx, dtype=np.int16):
    """dma_gather/scatter index layout: [128, n/16]; idx j at [j%16+16k, j//16]."""
    n = len(idx)
    out = np.zeros((P, n // 16), dtype=dtype)
    out[:16, :] = idx.astype(dtype).reshape(-1, 16).T
    out[16:, :] = np.tile(out[:16, :], (7, 1))
    return out


def _leaky(x):
    return np.maximum(x, 0.2 * x)


def host_prep(h, src, dst, W1, al1, ar1, W2, al2, ar2, W3, al3, ar3):
    f32 = np.float32
    h = np.asarray(h, f32)
    src = np.asarray(src, np.int64)
    dst = np.asarray(dst, np.int64)

    hp = np.zeros((NPAD, DIN), f32)
    hp[:N] = h

    # rho-space: local node n=(t*128+p) -> table row rho=p*49+t
    nn = np.arange(PC)
    rho = (nn % P) * WPC + (nn // P)             # node -> row
    rho_inv = np.empty(PC, np.int64)
    rho_inv[rho] = nn                            # row -> node
    g_of_dst = (dst // PC) * PC + rho[dst % PC]  # pi(dst): acc row in [0,NPAD)

    # layer-1 edge scores on host (x == h)
    wl1 = (np.asarray(W1, f32) @ np.asarray(al1, f32)).astype(f32)
    wr1 = (np.asarray(W1, f32) @ np.asarray(ar1, f32)).astype(f32)
    el1 = hp @ wl1
    er1 = hp @ wr1
    ex1_edge = np.exp(_leaky(el1[src] + er1[dst])).astype(f32)

    core_of = src // PC

    # ---- shared batch schedule: (half, round) with unique dst per round ----
    per_core = []
    maxr = 0
    for c in range(NCORES):
        sel = np.nonzero(core_of == c)[0]
        g = g_of_dst[sel]
        half = (g >= HALF).astype(np.int64)
        order = np.argsort(g, kind="stable")
        gs = g[order]
        run_start = np.where(np.r_[True, gs[1:] != gs[:-1]],
                             np.arange(len(gs)), -1)
        rnd_sorted = np.arange(len(gs)) - np.maximum.accumulate(run_start)
        rnd = np.empty(len(gs), np.int64)
        rnd[order] = rnd_sorted
        maxr = max(maxr, int(rnd.max()) + 1)
        per_core.append((sel, g, half, rnd))
    per_core = [(sel, g, half, (rnd + g) % maxr)
                for (sel, g, half, rnd) in per_core]

    counts = np.zeros((NCORES, 2, maxr), np.int64)
    for c, (sel, g, half, rnd) in enumerate(per_core):
        np.add.at(counts[c], (half, rnd), 1)
    size_hr = counts.max(axis=0)                 # shared sizes
    nch_hr = np.maximum(1, -(-size_hr // P))     # chunks per (half, round)

    batches = []                                 # (half, c0, c1) chunk ranges
    chunk0_hr = np.zeros((2, maxr), np.int64)
    off = 0
    for hf in range(2):
        for r in range(maxr):
            ncj = int(nch_hr[hf, r])
            chunk0_hr[hf, r] = off
            done = 0
            sz = int(size_hr[hf, r])
            while done < ncj:
                take = min(GMAX, ncj - done)
                valid = max(1, min(sz - done * P, take * P))
                batches.append((hf, off + done, off + done + take, valid))
                done += take
            off += ncj
    TOT = off

    in_maps = []
    for c, (sel, g, half, rnd) in enumerate(per_core):
        e_src_row = rho[src[sel] % PC]           # gather row in own table
        e_ex1 = ex1_edge[sel]

        slot = np.zeros(len(sel), np.int64)
        for hf in range(2):
            m_h = half == hf
            for r in range(maxr):
                m = np.nonzero(m_h & (rnd == r))[0]
                if len(m):
                    slot[m] = chunk0_hr[hf, r] * P + np.arange(len(m))

        S = TOT * P
        sidx = np.zeros(S, np.int64)
        scat = np.zeros(S, np.int64)
        dhi = np.zeros(S, np.int64)
        dlo = np.zeros(S, f32)
        mskv = np.zeros(S, f32)
        exv = np.zeros(S, f32)

        sidx[slot] = e_src_row
        scat[slot] = g - half * HALF
        dhi[slot] = g >> 6
        dlo[slot] = (g & 63).astype(f32)
        mskv[slot] = 1.0
        exv[slot] = e_ex1

        dm = np.stack([dlo, mskv], axis=1)       # [S, 2]

        hp_c = hp[c * PC:(c + 1) * PC]
        W2e = np.concatenate(
            [np.asarray(W2, f32),
             (np.asarray(W2, f32) @ np.asarray(al2, f32))[:, None],
             (np.asarray(W2, f32) @ np.asarray(ar2, f32))[:, None]], axis=1)
        W3e = np.concatenate(
            [np.asarray(W3, f32),
             (np.asarray(W3, f32) @ np.asarray(al3, f32))[:, None],
             (np.asarray(W3, f32) @ np.asarray(ar3, f32))[:, None]], axis=1)

        m = dict(
            hT=np.ascontiguousarray(hp_c[rho_inv].T),            # [256, PC]
            sidx=_wrap16(sidx),
            scat=_wrap16(scat),
            dhi=_wrap16(dhi),
            dm=np.ascontiguousarray(
                dm.reshape(TOT, P, 2).transpose(1, 0, 2).reshape(P, TOT * 2)),
            ex1=np.ascontiguousarray(exv.reshape(TOT, P).T),
            iota=np.tile(np.arange(P, dtype=f32), (P, 1)),
            W1=np.asarray(W1, f32),
            W2e=W2e, W3e=W3e,
        )
        in_maps.append(m)

    return dict(in_maps=in_maps, TOT=TOT, batches=batches, rho=rho,
                rho_inv=rho_inv, g_of_dst=g_of_dst)


def build_program(prep):
    import concourse.bacc as bacc
    import concourse.mybir as mybir
    import concourse.tile as tile
    from concourse import library_config

    f32 = mybir.dt.float32
    bf16 = mybir.dt.bfloat16
    i16 = mybir.dt.int16
    AF = mybir.ActivationFunctionType
    OP = mybir.AluOpType
    TOT = prep["TOT"]
    batches = prep["batches"]
    maxl = int(os.environ.get("GAT_MAXL", "3"))
    noedge = os.environ.get("GAT_NOEDGE")
    nors = os.environ.get("GAT_NORS")
    nopost = os.environ.get("GAT_NOPOST")
    noscat = os.environ.get("GAT_NOSCAT")
    noscale = os.environ.get("GAT_NOSCALE")

    nc = bacc.Bacc("TRN2", target_bir_lowering=False, debug=False,
                   num_devices=NCORES)

    hT_d = nc.dram_tensor("hT", [DIN, PC], f32, kind="ExternalInput")
    sidx_d = nc.dram_tensor("sidx", [P, TOT * 8], i16, kind="ExternalInput")
    scat_d = nc.dram_tensor("scat", [P, TOT * 8], i16, kind="ExternalInput")
    dhi_d = nc.dram_tensor("dhi", [P, TOT * 8], i16, kind="ExternalInput")
    dm_d = nc.dram_tensor("dm", [P, TOT * 2], f32, kind="ExternalInput")
    ex1_d = nc.dram_tensor("ex1", [P, TOT], f32, kind="ExternalInput")
    iota_d = nc.dram_tensor("iota", [P, P], f32, kind="ExternalInput")
    W1_d = nc.dram_tensor("W1", [DIN, DH], f32, kind="ExternalInput")
    W2e_d = nc.dram_tensor("W2e", [DH, DH + 2], f32, kind="ExternalInput")
    W3e_d = nc.dram_tensor("W3e", [DH, DOUT + 2], f32, kind="ExternalInput")
    out_d = nc.dram_tensor("out", [P, WPC * DOUT], f32, kind="ExternalOutput")

    LAY = {
        1: dict(dn=DH, tc=TC12, elc=DH, onec=DH + 1),
        2: dict(dn=DH, tc=TC12, elc=DH, onec=DH + 1),
        3: dict(dn=DOUT, tc=TC3, elc=DOUT, onec=DOUT + 1),
    }

    with tile.TileContext(nc) as tc:
        with (
            tc.tile_pool(name="sbP", bufs=1) as sbP,
            tc.tile_pool(name="sbG", bufs=2) as sbG,
            tc.tile_pool(name="sbE", bufs=2) as sbE,
            tc.tile_pool(name="sbS", bufs=3) as sbS,
            tc.tile_pool(name="psA", bufs=4, space="PSUM") as psA,
            tc.tile_pool(name="dram", bufs=1, space="DRAM") as dram,
        ):
            nc.gpsimd.load_library(library_config.mlp)

            iota = sbP.tile([P, P], f32, tag="iota")
            nc.sync.dma_start(iota[:], iota_d[:])
            ones = sbP.tile([P, 1], f32, tag="ones")
            nc.gpsimd.memset(ones[:], 1.0)
            zero = sbP.tile([P, 2688], f32, tag="zero")
            nc.vector.memset(zero[:], 0.0)

            W1t = [sbP.tile([P, DH], f32, tag=f"w1_{k}", name=f"w1_{k}")
                   for k in range(2)]
            for k in range(2):
                nc.sync.dma_start(W1t[k][:], W1_d[k * P:(k + 1) * P, :])
            W2t = sbP.tile([P, DH + 2], bf16, tag="w2")
            nc.gpsimd.dma_start(out=W2t[:], in_=W2e_d[:])
            W3t = sbP.tile([P, DOUT + 2], bf16, tag="w3")
            nc.gpsimd.dma_start(out=W3t[:], in_=W3e_d[:])

            for rep in range(int(os.environ.get("GAT_REPEAT", "1"))):
                tabs = {l: dram.tile([PC, LAY[l]["tc"]], f32, name=f"tab{l}_{rep}")
                        for l in (1, 2, 3)}
                accs = {l: dram.tile([NPAD, LAY[l]["tc"]], f32, name=f"acc{l}_{rep}")
                        for l in (1, 2, 3)}
                rs_out = {l: dram.tile([PC, LAY[l]["tc"]], f32,
                                       name=f"rso{l}_{rep}") for l in (1, 2, 3)}
                er_own = {l: dram.tile([PC], f32, name=f"ero{l}_{rep}")
                          for l in (2, 3)}
                er_full = {l: dram.tile([NPAD], f32, addr_space="Shared",
                                        name=f"erf{l}_{rep}") for l in (2, 3)}
                x_dram = {l: dram.tile([PC, DH], bf16, name=f"x{l}_{rep}")
                          for l in (2, 3)}

                for l in (1, 2, 3):
                    if l > maxl:
                        break
                    L = LAY[l]
                    dn, tcw, elc, onec = L["dn"], L["tc"], L["elc"], L["onec"]

                    # ---- zero the accumulator (28 DMAs) ----
                    zrows = 1792                 # 14 rows per partition
                    zcols = zrows * tcw // P
                    for k in range(NPAD // zrows):
                        nc.sync.dma_start(
                            accs[l][k * zrows:(k + 1) * zrows, :].rearrange(
                                "(p r) c -> p (r c)", p=P),
                            zero[:, 0:zcols])

                    # ---- projection: tab rows [f | el | ones] ----
                    if l >= 2:
                        xT = sbP.tile([P, PC], bf16, tag="xT")
                        nc.sync.dma_start(xT[:], x_dram[l][:], transpose=True)
                        er_stage = sbP.tile([P, WPC], f32, tag="er_stage")
                    WB = 3
                    ncols = dn + (2 if l >= 2 else 0)
                    for t0 in range(0, WPC, WB):
                        tw = min(WB, WPC - t0)
                        ps = psA.tile([P, WB * ncols], f32, space="PSUM", tag="pproj")
                        for j in range(tw):
                            t = t0 + j
                            sl = ps[:, j * ncols:(j + 1) * ncols]
                            if l == 1:
                                for k in range(2):
                                    xt = sbS.tile([P, P], f32, tag="hTk")
                                    nc.sync.dma_start(
                                        xt[:], hT_d[k * P:(k + 1) * P,
                                                    t * P:(t + 1) * P])
                                    nc.tensor.matmul(sl, xt[:], W1t[k][:],
                                                     start=(k == 0), stop=(k == 1))
                            else:
                                Wt = W2t if l == 2 else W3t
                                nc.tensor.matmul(
                                    sl, xT[:, t * P:(t + 1) * P], Wt[:],
                                    start=True, stop=True)
                        stage = sbS.tile([P, WB, tcw], f32, tag="tstage")
                        ps3 = ps[:].rearrange("p (w c) -> p w c", c=ncols)
                        if l >= 2:
                            nc.scalar.copy(stage[:, 0:tw, 0:dn + 1],
                                           ps3[:, 0:tw, 0:dn + 1])
                            nc.vector.tensor_copy(
                                er_stage[:, t0:t0 + tw],
                                ps3[:, 0:tw, dn + 1:dn + 2].rearrange(
                                    "p w u -> p (w u)"))
                        else:
                            nc.scalar.copy(stage[:, 0:tw, 0:dn],
                                           ps3[:, 0:tw, 0:dn])
                        nc.vector.tensor_copy(
                            stage[:, 0:tw, onec:onec + 1],
                            ones[:].rearrange("p (u v) -> p u v", u=1).to_broadcast(
                                [P, tw, 1]))
                        nc.sync.dma_start(
                            tabs[l][t0 * P:(t0 + tw) * P, :].rearrange(
                                "(w p) c -> p w c", p=P),
                            stage[:, 0:tw, :])

                    if l >= 2:
                        nc.sync.dma_start(
                            er_own[l][:].rearrange("(w p) -> p w", p=P),
                            er_stage[:])
                        nc.gpsimd.collective_compute(
                            "AllGather", mybir.AluOpType.bypass,
                            ins=[er_own[l][:]], outs=[er_full[l][:]],
                            replica_groups=[list(range(NCORES))])

                    # ---- edge batches: gather -> ex -> scale -> scatter ----
                    for (hf, c0, c1, nv) in (batches if not noedge else []):
                        cb = c1 - c0
                        si = sbS.tile([P, GMAX * 8], i16, tag="si")
                        nc.sync.dma_start(si[:, 0:cb * 8], sidx_d[:, c0 * 8:c1 * 8])
                        G = sbG.tile([P, GMAX, tcw], f32, tag="G")
                        nc.gpsimd.dma_gather(
                            G[:, 0:cb, :], tabs[l][:], si[:, 0:cb * 8],
                            cb * P, cb * P, tcw, single_packet=False)
                        if l == 1:
                            exm = sbS.tile([P, GMAX], f32, tag="exm")
                            nc.sync.dma_start(exm[:, 0:cb], ex1_d[:, c0:c1])
                        else:
                            di = sbS.tile([P, GMAX * 8], i16, tag="di")
                            nc.sync.dma_start(di[:, 0:cb * 8],
                                              dhi_d[:, c0 * 8:c1 * 8])
                            erG = sbE.tile([P, GMAX, 64], f32, tag="erG")
                            nc.gpsimd.dma_gather(
                                erG[:, 0:cb, :],
                                er_full[l][:].rearrange("(r k) -> r k", k=64),
                                di[:, 0:cb * 8], cb * P, cb * P, 64,
                                single_packet=False)
                            dm = sbS.tile([P, GMAX, 2], f32, tag="dm")
                            nc.sync.dma_start(
                                dm[:, 0:cb, :].rearrange("p c k -> p (c k)"),
                                dm_d[:, c0 * 2:c1 * 2])
                            msk = sbE.tile([P, GMAX, 64], f32, tag="msk")
                            nc.vector.tensor_tensor(
                                out=msk[:, 0:cb, :],
                                in0=iota[:, 0:64].rearrange(
                                    "p (u d) -> p u d", u=1).to_broadcast([P, cb, 64]),
                                in1=dm[:, 0:cb, 0:1].to_broadcast([P, cb, 64]),
                                op=OP.is_equal)
                            nc.vector.tensor_tensor(
                                out=msk[:, 0:cb, :], in0=msk[:, 0:cb, :],
                                in1=erG[:, 0:cb, :], op=OP.mult)
                            ere = sbS.tile([P, GMAX], f32, tag="ere")
                            nc.vector.tensor_reduce(
                                out=ere[:, 0:cb], in_=msk[:, 0:cb, :],
                                op=OP.add, axis=mybir.AxisListType.X)
                            sc = sbS.tile([P, GMAX], f32, tag="sc")
                            nc.vector.tensor_tensor(
                                out=sc[:, 0:cb], in0=ere[:, 0:cb],
                                in1=G[:, 0:cb, elc:elc + 1].rearrange(
                                    "p c u -> p (c u)"), op=OP.add)
                            nc.vector.scalar_tensor_tensor(
                                out=sc[:, 0:cb], in0=sc[:, 0:cb], scalar=0.2,
                                in1=sc[:, 0:cb], op0=OP.mult, op1=OP.max)
                            exm = sbS.tile([P, GMAX], f32, tag="exm")
                            nc.scalar.activation(exm[:, 0:cb], sc[:, 0:cb], AF.Exp)
                            nc.vector.tensor_tensor(
                                out=exm[:, 0:cb], in0=exm[:, 0:cb],
                                in1=dm[:, 0:cb, 1:2].rearrange(
                                    "p c u -> p (c u)"), op=OP.mult)
                        if not noscale:
                            nc.vector.tensor_tensor(
                                out=G[:, 0:cb, :], in0=G[:, 0:cb, :],
                                in1=exm[:, 0:cb].rearrange("p (c v) -> p c v", v=1)
                                .to_broadcast([P, cb, tcw]), op=OP.mult)
                        if not noscat:
                            s2 = sbS.tile([P, GMAX * 8], i16, tag="s2")
                            nc.sync.dma_start(s2[:, 0:cb * 8],
                                              scat_d[:, c0 * 8:c1 * 8])
                            nc.gpsimd.dma_scatter_add(
                                accs[l][hf * HALF:(hf + 1) * HALF, :],
                                G[:, 0:cb, :], s2[:, 0:cb * 8],
                                nv, nv, tcw)

                    # ---- reduce-scatter + post ----
                    if not nors:
                        nc.gpsimd.collective_compute(
                            "ReduceScatter", mybir.AluOpType.add,
                            ins=[accs[l][:]], outs=[rs_out[l][:]],
                            replica_groups=[list(range(NCORES))])
                    if nopost:
                        continue
                    xp = sbP.tile([P, WPC, tcw], f32, tag="xp")
                    nc.sync.dma_start(
                        xp[:].rearrange("p t c -> p (t c)"),
                        rs_out[l][:].rearrange("(p t) c -> p (t c)", p=P))
                    zc = sbS.tile([P, WPC], f32, tag="zc")
                    nc.vector.tensor_scalar(
                        out=zc[:], in0=xp[:, :, onec:onec + 1].rearrange(
                            "p t u -> p (t u)"),
                        scalar1=1e-9, scalar2=None, op0=OP.max)
                    zr = sbS.tile([P, WPC], f32, tag="zr")
                    nc.vector.reciprocal(zr[:], zc[:])
                    if l < 3:
                        xb = sbP.tile([P, WPC, dn], bf16, tag="pq1")
                        nc.vector.tensor_tensor(
                            out=xb[:], in0=xp[:, :, 0:dn],
                            in1=zr[:].rearrange("p (t v) -> p t v", v=1).to_broadcast(
                                [P, WPC, dn]), op=OP.mult)
                        xb2 = sbP.tile([P, WPC, dn], bf16, tag="pq2")
                        nc.vector.tensor_scalar(
                            out=xb2[:], in0=xb[:], scalar1=0.0, scalar2=None,
                            op0=OP.max)
                        nc.sync.dma_start(
                            x_dram[l + 1][:].rearrange(
                                "(p t) c -> p (t c)", p=P),
                            xb2[:].rearrange("p t c -> p (t c)"))
                    else:
                        xs = sbP.tile([P, WPC, DOUT], f32, tag="pq1")
                        nc.vector.tensor_tensor(
                            out=xs[:], in0=xp[:, :, 0:DOUT],
                            in1=zr[:].rearrange("p (t v) -> p t v", v=1).to_broadcast(
                                [P, WPC, DOUT]), op=OP.mult)
                        mx = sbS.tile([P, WPC], f32, tag="mx")
                        nc.vector.tensor_reduce(
                            out=mx[:], in_=xs[:], op=OP.max,
                            axis=mybir.AxisListType.X)
                        xm = sbP.tile([P, WPC, DOUT], f32, tag="pq2")
                        nc.vector.tensor_tensor(
                            out=xm[:], in0=xs[:],
                            in1=mx[:].rearrange("p (t v) -> p t v", v=1).to_broadcast(
                                [P, WPC, DOUT]), op=OP.subtract)
                        ee = sbP.tile([P, WPC, DOUT], f32, tag="pq3")
                        nc.scalar.activation(ee[:], xm[:], AF.Exp)
                        se = sbS.tile([P, WPC], f32, tag="se")
                        nc.vector.tensor_reduce(
                            out=se[:], in_=ee[:], op=OP.add,
                            axis=mybir.AxisListType.X)
                        ls = sbS.tile([P, WPC], f32, tag="ls")
                        nc.scalar.activation(ls[:], se[:], AF.Ln)
                        fo = sbP.tile([P, WPC, DOUT], f32, tag="pq4")
                        nc.vector.tensor_tensor(
                            out=fo[:], in0=xm[:],
                            in1=ls[:].rearrange("p (t v) -> p t v", v=1).to_broadcast(
                                [P, WPC, DOUT]), op=OP.subtract)
                        nc.sync.dma_start(
                            out_d[:], fo[:].rearrange("p t c -> p (t c)"))

    nc.compile()
    return nc


def _unpermute(raw):
    """raw [P, WPC*DOUT] (row p*49+t) -> node-order [PC, DOUT]."""
    byrow = raw.reshape(P * WPC, DOUT)
    nn = np.arange(PC)
    return byrow[(nn % P) * WPC + (nn // P)]


def kernel(**inputs):
    from concourse.bass_utils import run_bass_kernel_spmd

    prep = host_prep(**inputs)
    nc = build_program(prep)
    res = run_bass_kernel_spmd(nc, prep["in_maps"], core_ids=list(range(NCORES)))
    full = np.concatenate(
        [_unpermute(np.asarray(res.results[c]["out"])) for c in range(NCORES)],
        axis=0)
    return np.ascontiguousarray(full[:N]).astype(np.float32)

